# revision 11
# baseline (speedup 1.0000x reference)
"""Hamming-distance kernel for Trainium2 (8 NeuronCores, SPMD).

out[n, m] = mean_d(x[n, d] != y[m, d]),  x: (8192, 256), y: (8192, 256),
values are small integers 0..7 stored as float32.

Formulation: categorical equality as a +-1 Hadamard-code GEMM.  Each value
c in {0..7} maps to the 7 non-constant entries of row c of the 8x8
Hadamard matrix: had_j(c) = (-1)^popcount(c & k_j), k_j in {1..7}.  Rows
satisfy <h(a), h(b)> = 8*[a==b] - 1, so with dot[n,m] over K = 7*256 =
1792 features:  eq = (dot + 256)/8  and  out = 1 - eq/256 = 0.875 -
dot/2048.  All code values are +-1 (exact in fp8e4), PSUM accumulates in
fp32 (|dot| <= 1792 << 2^24), and 0.875 - dot*2^-11 is exact binary
arithmetic, so the result is bit-exact.  K = 7 per dim is the provable
minimum embedding for exact categorical equality (vs 8 for one-hot).

Sharding: x rows split across 8 cores (1024 rows each), y replicated.
Each core computes a (1024, 8192) slice of the output.

Pipeline (v2 — m-chunk-outer, targets the 216 ns/matmul fp8 DoubleRow
issue rate end to end):
  - Raw values ship as fp8e4 (0..7 exact): x shard 256 KB, y 2 MB, so
    input DMA is off the critical path.  y chunks 0-1 also ship as a
    small separate tensor so their encode can start ~6 us in.
  - PE p-state warmup: ~24 dummy matmuls into a scratch PSUM bank while
    DMAs land, plus a dummy Sign to preload the ACT table.
  - Encode chains (cast -> bitwise AND -> Sign / products) emit codes in
    matmul kp order and are interleaved into the matmul issue stream as
    generators, so each chunk's codes appear one pass ahead of use.
  - Main loop over 16 m-chunks; per chunk two half-passes of 4 n-tiles
    (kp outer, psum bank per n-tile).  Half-passes double the PSUM bank
    reuse distance so ACT evictions never stall the PE.
"""

import numpy as np
import ml_dtypes

import concourse.bacc as bacc
import concourse.mybir as mybir
import concourse.tile as tile
from concourse.bass_utils import run_bass_kernel_spmd

# Problem dims (hardcoded per contract).
N, M, D, C = 8192, 8192, 256, 8
N_CORES = 8
N_SH = N // N_CORES  # 1024 x-rows per core

P = 128
D_HALVES = D // P  # 2
N_CODES = 7  # +-1 Hadamard code length per dim
KSUB = N_CODES * D_HALVES  # 14 k-subtiles of 128 features -> K = 1792
K_PAIRS = KSUB // 2  # 7 DoubleRow pairs (256 contracted per matmul)
M_CHUNK = 512  # output free-dim tile (one PSUM bank of f32)
M_CHUNKS = M // M_CHUNK  # 16
N_TILES = N_SH // P  # 8
X_CH = N_SH // M_CHUNK  # 2 encode chunks of 512 x-columns

FP8 = mybir.dt.float8e4
F32 = mybir.dt.float32
BF16 = mybir.dt.bfloat16
I32 = mybir.dt.int32
ALU = mybir.AluOpType
ACTF = mybir.ActivationFunctionType
DR = mybir.MatmulPerfMode.DoubleRow

N_WARM = 20  # dummy matmuls for PE p-state ramp during the DMA head
YE_RING = 3  # encoded-y chunk ring


def _enc_chain(nc, tmp_pool, slot, raw, w, ts_engine, biases, act_cast=False):
    """Generator emitting the 7 +-1 code tiles for one (chunk, half) in
    kp order; yields after each engine op so the driver can interleave
    program order with the matmul stream.

    slot(j) -> dst AP for code j (kp order = masks [1,2,4,3,5,6,7]);
    raw = fp8 source AP [P, w]; ts_engine handles the int bit extracts.
    act_cast moves the fp8->i32 cast to the Scalar engine (idle during the
    head, where DVE is the bottleneck).
    """
    b05, b15, b35 = biases
    vi = tmp_pool.tile([P, w], I32, name="enc_vi")
    if act_cast:
        nc.scalar.activation(vi[:], raw, ACTF.Copy, bias=0.0, scale=1.0)
    else:
        nc.vector.tensor_copy(vi[:], raw)
    yield
    t0 = tmp_pool.tile([P, w], I32, name="enc_t0")
    ts_engine.tensor_scalar(
        out=t0[:], in0=vi[:], scalar1=1, scalar2=None, op0=ALU.bitwise_and
    )
    yield
    s1 = slot(0)
    nc.scalar.activation(s1, t0[:], ACTF.Sign, bias=b05[:], scale=-1.0)
    yield
    u = tmp_pool.tile([P, w], I32, name="enc_u")
    ts_engine.tensor_scalar(
        out=u[:], in0=vi[:], scalar1=2, scalar2=None, op0=ALU.bitwise_and
    )
    yield
    s2 = slot(1)
    nc.scalar.activation(s2, u[:], ACTF.Sign, bias=b15[:], scale=-1.0)
    yield
    s4 = slot(2)
    nc.scalar.activation(s4, raw, ACTF.Sign, bias=b35[:], scale=-1.0)
    yield
    s3 = slot(3)
    nc.vector.tensor_tensor(s3, s1, s2, ALU.mult)
    yield
    s5 = slot(4)
    nc.vector.tensor_tensor(s5, s1, s4, ALU.mult)
    yield
    s6 = slot(5)
    nc.vector.tensor_tensor(s6, s2, s4, ALU.mult)
    yield
    s7 = slot(6)
    nc.vector.tensor_tensor(s7, s3, s4, ALU.mult)
    yield


def _drain(chains, steps):
    """Advance up to `steps` ops round-robin across pending chains."""
    n = 0
    while chains and n < steps:
        ch = chains[0]
        try:
            next(ch)
            n += 1
        except StopIteration:
            chains.pop(0)
            continue
        chains.append(chains.pop(0))
    return chains


def _build_bass():
    nc = bacc.Bacc(
        "TRN2", target_bir_lowering=False, debug=False, num_devices=N_CORES
    )

    # fp8 raw values, feature dim on partitions: t[p, h, i] = v[i, h*128+p]
    xr_d = nc.dram_tensor("xr", [P, D_HALVES, N_SH], FP8, kind="ExternalInput")
    y01_d = nc.dram_tensor(
        "y01", [P, D_HALVES, 2 * M_CHUNK], FP8, kind="ExternalInput"
    )
    yf_d = nc.dram_tensor("yf", [P, D_HALVES, M], FP8, kind="ExternalInput")
    # Blocked output: block (n, mc) is one contiguous 128x512 f32 region.
    # bf16 output (upcast on host): halves HBM write traffic; adds <0.4%
    # rel error against a 2e-2 tolerance.
    out_d = nc.dram_tensor(
        "out", [N_TILES, M_CHUNKS, P, M_CHUNK], BF16, kind="ExternalOutput"
    )

    with tile.TileContext(nc) as tc:
        with (
            tc.tile_pool(name="warm", bufs=1) as warm_pool,
            tc.tile_pool(name="xraw", bufs=1) as xraw_pool,
            tc.tile_pool(name="y01raw", bufs=1) as y01_pool,
            tc.tile_pool(name="yfraw", bufs=1) as yf_pool,
            tc.tile_pool(name="xe", bufs=1) as xe_pool,
            tc.tile_pool(name="ye", bufs=YE_RING) as ye_pool,
            tc.tile_pool(name="tmp", bufs=6) as tmp_pool,
            tc.tile_pool(name="out", bufs=8) as out_pool,
            tc.tile_pool(name="psum", bufs=8, space="PSUM") as psum_pool,
        ):
            # ---- input DMAs: all on Sync, urgency order, so the bulk-y
            # packets queue behind the head-critical y01/xr packets ----
            y01 = y01_pool.tile([P, D_HALVES, 2 * M_CHUNK], FP8)
            nc.sync.dma_start(y01[:], y01_d[:, :, :])
            xraw = xraw_pool.tile([P, D_HALVES, N_SH], FP8)
            nc.sync.dma_start(xraw[:], xr_d[:, :, :])
            yf = yf_pool.tile([P, D_HALVES, M], FP8)
            nc.sync.dma_start(yf[:, :, : M // 2], yf_d[:, :, : M // 2])
            nc.sync.dma_start(yf[:, :, M // 2 :], yf_d[:, :, M // 2 :])

            # ---- bias constants for ACT Sign ----
            biases = []
            for val in (0.5, 1.5, 3.5):
                b = tmp_pool.tile([P, 1], F32, name=f"bias_{val}", bufs=1)
                nc.vector.memset(b[:], val)
                biases.append(b)

            # ---- PE warmup + ACT Sign-table preload (runs during DMAs) ----
            dw = warm_pool.tile([P, D_HALVES, M_CHUNK], FP8)
            nc.gpsimd.memset(dw[:], 1.0)
            # Table preload reads a bias tile (ready ~4.7us) so the Sign
            # table is resident before the first encode sign.
            wsig = warm_pool.tile([P, 1], F32)
            nc.scalar.activation(
                wsig[:], biases[1][:], ACTF.Sign, bias=biases[0][:], scale=-1.0
            )
            # Round-robin all 8 banks so WAW sem chains never gate the issue
            # rate; sustains the 216 ns/matmul stream through the DMA head.
            wpsums = [
                psum_pool.tile([P, M_CHUNK], F32, name="psum") for _ in range(8)
            ]
            for i in range(N_WARM):
                nc.tensor.matmul(
                    wpsums[i % 8][:], dw[:, :, :P], dw[:], start=True,
                    stop=True, perf_mode=DR,
                )

            # ---- encoded-code tiles ----
            xe = xe_pool.tile([P, KSUB, N_SH], FP8)

            def x_slot(h):
                return lambda j: xe[:, 2 * j + h, :]

            ye_tiles = [None] * M_CHUNKS

            def start_y_chunk(c):
                yt = ye_pool.tile([P, KSUB, M_CHUNK], FP8, name="ye")
                ye_tiles[c] = yt
                if c < 2:
                    raw = y01[:, :, c * M_CHUNK : (c + 1) * M_CHUNK]
                else:
                    raw = yf[:, :, c * M_CHUNK : (c + 1) * M_CHUNK]
                chains = []
                for h in range(D_HALVES):
                    slot = lambda j, h=h: yt[:, 2 * j + h, :]
                    chains.append(
                        _enc_chain(
                            nc, tmp_pool, slot, raw[:, h, :], M_CHUNK,
                            nc.vector, biases,
                        )
                    )
                return chains

            # ---- head: x + y0 first (feed pass 0), then y1; x chains
            # lead the rotation since they are 2x wider ----
            head = []
            for h in range(D_HALVES):
                head.append(
                    _enc_chain(
                        nc, tmp_pool, x_slot(h), xraw[:, h, :], N_SH,
                        nc.vector, biases, act_cast=True,
                    )
                )
            head += start_y_chunk(0)
            _drain(head, 10_000)
            _drain(start_y_chunk(1), 10_000)

            # ---- main loop over m-chunks ----
            pending = []
            for mc in range(M_CHUNKS):
                if mc + 2 < M_CHUNKS:
                    pending += start_y_chunk(mc + 2)
                yt = ye_tiles[mc]
                for half in range(2):
                    psums = [
                        psum_pool.tile([P, M_CHUNK], F32, name="psum")
                        for _ in range(4)
                    ]
                    for kp in range(K_PAIRS):
                        for nn in range(4):
                            n = 4 * half + nn
                            nc.tensor.matmul(
                                psums[nn][:],
                                xe[:, 2 * kp : 2 * kp + 2, n * P : (n + 1) * P],
                                yt[:, 2 * kp : 2 * kp + 2, :],
                                start=(kp == 0),
                                stop=(kp == K_PAIRS - 1),
                                perf_mode=DR,
                            )
                        pending = _drain(pending, 2)
                    for nn in range(4):
                        n = 4 * half + nn
                        ot = out_pool.tile([P, M_CHUNK], BF16, name="ot")
                        # out = 0.875 - dot/2048  (exact before bf16 round)
                        if mc == M_CHUNKS - 1 and nn % 2 == 1:
                            # DVE is idle by the tail; splitting the final
                            # evictions drains the last blocks sooner.
                            nc.vector.tensor_scalar(
                                out=ot[:], in0=psums[nn][:],
                                scalar1=-1.0 / 2048.0, scalar2=0.875,
                                op0=ALU.mult, op1=ALU.add,
                            )
                        else:
                            nc.scalar.activation(
                                ot[:], psums[nn][:], ACTF.Copy,
                                bias=0.875, scale=-1.0 / 2048.0,
                            )
                        nc.sync.dma_start(out_d[n, mc], ot[:])
                _drain(pending, 4)
    nc.compile()
    return nc


_NC_CACHE = {}


def _get_nc():
    if "nc" not in _NC_CACHE:
        _NC_CACHE["nc"] = _build_bass()
    return _NC_CACHE["nc"]


def _pack_features(t: np.ndarray) -> np.ndarray:
    """(rows, 256) values -> fp8 [128, 2, rows]: out[p, h, i] = t[i, 128h+p]."""
    tt = np.ascontiguousarray(t.T)  # (256, rows)
    return np.ascontiguousarray(
        tt.reshape(D_HALVES, P, t.shape[0]).transpose(1, 0, 2)
    ).astype(ml_dtypes.float8_e4m3fn)


def _make_in_maps(x: np.ndarray, y: np.ndarray):
    yr = _pack_features(y)  # [128, 2, 8192] fp8
    y01 = np.ascontiguousarray(yr[:, :, : 2 * M_CHUNK])
    in_maps = []
    for i in range(N_CORES):
        xr = _pack_features(x[i * N_SH : (i + 1) * N_SH])  # [128, 2, 1024]
        in_maps.append({"xr": xr, "y01": y01, "yf": yr})
    return in_maps


def _deblock(blocked: np.ndarray) -> np.ndarray:
    # (N_TILES, M_CHUNKS, P, M_CHUNK) -> (N_SH, M)
    return np.ascontiguousarray(
        blocked.transpose(0, 2, 1, 3).reshape(N_SH, M)
    )


def kernel(x: np.ndarray, y: np.ndarray, _trace: bool = False):
    x = np.asarray(x, dtype=np.float32)
    y = np.asarray(y, dtype=np.float32)
    assert x.shape == (N, D) and y.shape == (M, D)

    nc = _get_nc()
    in_maps = _make_in_maps(x, y)
    res = run_bass_kernel_spmd(
        nc, in_maps, core_ids=list(range(N_CORES)), trace=_trace
    )
    out = np.concatenate(
        [_deblock(np.asarray(r["out"]).astype(np.float32)) for r in res.results],
        axis=0,
    )
    if _trace:
        return out, res
    return out


# revision 12
# speedup vs baseline: 1.2088x; 1.2088x over previous
"""Hamming-distance kernel for Trainium2 (8 NeuronCores, SPMD).

out[n, m] = mean_d(x[n, d] != y[m, d]),  x: (8192, 256), y: (8192, 256),
values are small integers 0..7 stored as float32.

Formulation: categorical equality as a +-1 Hadamard-code GEMM.  Each value
c in {0..7} maps to the 7 non-constant entries of row c of the 8x8
Hadamard matrix: had_j(c) = (-1)^popcount(c & k_j), k_j in {1..7}.  Rows
satisfy <h(a), h(b)> = 8*[a==b] - 1, so with dot[n,m] over K = 7*256 =
1792 features:  eq = (dot + 256)/8  and  out = 1 - eq/256 = 0.875 -
dot/2048.  All code values are +-1 (exact in fp8e4), PSUM accumulates in
fp32 (|dot| <= 1792 << 2^24), and 0.875 - dot*2^-11 is exact binary
arithmetic, so the result is bit-exact.  K = 7 per dim is the provable
minimum embedding for exact categorical equality (vs 8 for one-hot).

Sharding: x rows split across 8 cores (1024 rows each), y replicated.
Each core computes a (1024, 8192) slice of the output.

Pipeline (v2 — m-chunk-outer, targets the 216 ns/matmul fp8 DoubleRow
issue rate end to end):
  - Raw values ship as fp8e4 (0..7 exact): x shard 256 KB, y 2 MB, so
    input DMA is off the critical path.  y chunks 0-1 also ship as a
    small separate tensor so their encode can start ~6 us in.
  - PE p-state warmup: ~24 dummy matmuls into a scratch PSUM bank while
    DMAs land, plus a dummy Sign to preload the ACT table.
  - Encode chains (cast -> bitwise AND -> Sign / products) emit codes in
    matmul kp order and are interleaved into the matmul issue stream as
    generators, so each chunk's codes appear one pass ahead of use.
  - Main loop over 16 m-chunks; per chunk two half-passes of 4 n-tiles
    (kp outer, psum bank per n-tile).  Half-passes double the PSUM bank
    reuse distance so ACT evictions never stall the PE.
"""

import numpy as np
import ml_dtypes

import concourse.bacc as bacc
import concourse.mybir as mybir
import concourse.tile as tile
from concourse.bass_utils import run_bass_kernel_spmd

# Problem dims (hardcoded per contract).
N, M, D, C = 8192, 8192, 256, 8
N_CORES = 8
N_SH = N // N_CORES  # 1024 x-rows per core

P = 128
D_HALVES = D // P  # 2
N_CODES = 7  # +-1 Hadamard code length per dim
KSUB = N_CODES * D_HALVES  # 14 k-subtiles of 128 features -> K = 1792
K_PAIRS = KSUB // 2  # 7 DoubleRow pairs (256 contracted per matmul)
M_CHUNK = 512  # output free-dim tile (one PSUM bank of f32)
M_CHUNKS = M // M_CHUNK  # 16
N_TILES = N_SH // P  # 8
X_CH = N_SH // M_CHUNK  # 2 encode chunks of 512 x-columns

FP8 = mybir.dt.float8e4
F32 = mybir.dt.float32
BF16 = mybir.dt.bfloat16
I32 = mybir.dt.int32
ALU = mybir.AluOpType
ACTF = mybir.ActivationFunctionType
DR = mybir.MatmulPerfMode.DoubleRow

N_WARM = 16  # dummy matmuls for PE p-state ramp during the DMA head
YE_RING = 3  # encoded-y chunk ring


def _enc_chain(nc, tmp_pool, slot, raw, w, ts_engine, biases, act_cast=False):
    """Generator emitting the 7 +-1 code tiles for one (chunk, half) in
    kp order; yields after each engine op so the driver can interleave
    program order with the matmul stream.

    slot(j) -> dst AP for code j (kp order = masks [1,2,4,3,5,6,7]);
    raw = fp8 source AP [P, w]; ts_engine handles the int bit extracts.
    act_cast moves the fp8->i32 cast to the Scalar engine (idle during the
    head, where DVE is the bottleneck).
    """
    b05, b15, b35 = biases
    vi = tmp_pool.tile([P, w], I32, name="enc_vi")
    if act_cast:
        nc.scalar.activation(vi[:], raw, ACTF.Copy, bias=0.0, scale=1.0)
    else:
        nc.vector.tensor_copy(vi[:], raw)
    yield
    t0 = tmp_pool.tile([P, w], I32, name="enc_t0")
    ts_engine.tensor_scalar(
        out=t0[:], in0=vi[:], scalar1=1, scalar2=None, op0=ALU.bitwise_and
    )
    yield
    s1 = slot(0)
    nc.scalar.activation(s1, t0[:], ACTF.Sign, bias=b05[:], scale=-1.0)
    yield
    u = tmp_pool.tile([P, w], I32, name="enc_u")
    ts_engine.tensor_scalar(
        out=u[:], in0=vi[:], scalar1=2, scalar2=None, op0=ALU.bitwise_and
    )
    yield
    s2 = slot(1)
    nc.scalar.activation(s2, u[:], ACTF.Sign, bias=b15[:], scale=-1.0)
    yield
    s4 = slot(2)
    nc.scalar.activation(s4, raw, ACTF.Sign, bias=b35[:], scale=-1.0)
    yield
    s3 = slot(3)
    nc.vector.tensor_tensor(s3, s1, s2, ALU.mult)
    yield
    s5 = slot(4)
    nc.vector.tensor_tensor(s5, s1, s4, ALU.mult)
    yield
    s6 = slot(5)
    nc.vector.tensor_tensor(s6, s2, s4, ALU.mult)
    yield
    s7 = slot(6)
    nc.vector.tensor_tensor(s7, s3, s4, ALU.mult)
    yield


def _drain(chains, steps):
    """Advance up to `steps` ops round-robin across pending chains."""
    n = 0
    while chains and n < steps:
        ch = chains[0]
        try:
            next(ch)
            n += 1
        except StopIteration:
            chains.pop(0)
            continue
        chains.append(chains.pop(0))
    return chains


def _build_bass():
    nc = bacc.Bacc(
        "TRN2", target_bir_lowering=False, debug=False, num_devices=N_CORES
    )

    # fp8 raw values, feature dim on partitions: t[p, h, i] = v[i, h*128+p]
    xr_d = nc.dram_tensor("xr", [P, D_HALVES, N_SH], FP8, kind="ExternalInput")
    y01_d = nc.dram_tensor(
        "y01", [P, D_HALVES, 2 * M_CHUNK], FP8, kind="ExternalInput"
    )
    yf_d = nc.dram_tensor("yf", [P, D_HALVES, M], FP8, kind="ExternalInput")
    # Blocked output: block (n, mc) is one contiguous 128x512 f32 region.
    # bf16 output (upcast on host): halves HBM write traffic; adds <0.4%
    # rel error against a 2e-2 tolerance.
    out_d = nc.dram_tensor(
        "out", [N_TILES, M_CHUNKS, P, M_CHUNK], BF16, kind="ExternalOutput"
    )

    with tile.TileContext(nc) as tc:
        with (
            tc.tile_pool(name="warm", bufs=1) as warm_pool,
            tc.tile_pool(name="xraw", bufs=1) as xraw_pool,
            tc.tile_pool(name="y01raw", bufs=1) as y01_pool,
            tc.tile_pool(name="yfraw", bufs=1) as yf_pool,
            tc.tile_pool(name="xe", bufs=1) as xe_pool,
            tc.tile_pool(name="ye", bufs=YE_RING) as ye_pool,
            tc.tile_pool(name="tmp", bufs=6) as tmp_pool,
            tc.tile_pool(name="out", bufs=8) as out_pool,
            tc.tile_pool(name="psum", bufs=8, space="PSUM") as psum_pool,
        ):
            # ---- bias constants for ACT Sign ----
            biases = []
            for val in (0.5, 1.5, 3.5):
                b = tmp_pool.tile([P, 1], F32, name=f"bias_{val}", bufs=1)
                nc.vector.memset(b[:], val)
                biases.append(b)

            # ---- PE warmup + ACT Sign-table preload (runs during DMAs) ----
            dw = warm_pool.tile([P, D_HALVES, M_CHUNK], FP8)
            nc.gpsimd.memset(dw[:], 1.0)
            # Table preload reads a bias tile (ready ~4.7us) so the Sign
            # table is resident before the first encode sign.
            wsig = warm_pool.tile([P, 1], F32)
            nc.scalar.activation(
                wsig[:], biases[1][:], ACTF.Sign, bias=biases[0][:], scale=-1.0
            )
            # Round-robin all 8 banks so WAW sem chains never gate the issue
            # rate; sustains the 216 ns/matmul stream through the DMA head.
            wpsums = [
                psum_pool.tile([P, M_CHUNK], F32, name="psum") for _ in range(8)
            ]
            for i in range(N_WARM):
                nc.tensor.matmul(
                    wpsums[i % 8][:], dw[:, :, :P], dw[:], start=True,
                    stop=True, perf_mode=DR,
                )

            # ---- input DMAs: all on Sync, urgency order, so the bulk-y
            # packets queue behind the head-critical y01/xr packets ----
            y01 = y01_pool.tile([P, D_HALVES, 2 * M_CHUNK], FP8)
            nc.sync.dma_start(y01[:], y01_d[:, :, :])
            xraw = xraw_pool.tile([P, D_HALVES, N_SH], FP8)
            nc.sync.dma_start(xraw[:], xr_d[:, :, :])
            yf = yf_pool.tile([P, D_HALVES, M], FP8)
            nc.sync.dma_start(yf[:, :, : M // 2], yf_d[:, :, : M // 2])
            nc.sync.dma_start(yf[:, :, M // 2 :], yf_d[:, :, M // 2 :])

            # ---- encoded-code tiles ----
            xe = xe_pool.tile([P, KSUB, N_SH], FP8)

            def x_slot(h):
                return lambda j: xe[:, 2 * j + h, :]

            ye_tiles = [None] * M_CHUNKS

            def start_y_chunk(c):
                yt = ye_pool.tile([P, KSUB, M_CHUNK], FP8, name="ye")
                ye_tiles[c] = yt
                if c < 2:
                    raw = y01[:, :, c * M_CHUNK : (c + 1) * M_CHUNK]
                else:
                    raw = yf[:, :, c * M_CHUNK : (c + 1) * M_CHUNK]
                chains = []
                for h in range(D_HALVES):
                    slot = lambda j, h=h: yt[:, 2 * j + h, :]
                    chains.append(
                        _enc_chain(
                            nc, tmp_pool, slot, raw[:, h, :], M_CHUNK,
                            nc.vector, biases,
                        )
                    )
                return chains

            # ---- head: y0 + x first (feed pass 0), then y1 ----
            head = []
            head += start_y_chunk(0)
            for h in range(D_HALVES):
                head.append(
                    _enc_chain(
                        nc, tmp_pool, x_slot(h), xraw[:, h, :], N_SH,
                        nc.vector, biases, act_cast=True,
                    )
                )
            _drain(head, 10_000)
            _drain(start_y_chunk(1), 10_000)

            # ---- main loop over m-chunks ----
            pending = []
            for mc in range(M_CHUNKS):
                if mc + 2 < M_CHUNKS:
                    pending += start_y_chunk(mc + 2)
                yt = ye_tiles[mc]
                for half in range(2):
                    psums = [
                        psum_pool.tile([P, M_CHUNK], F32, name="psum")
                        for _ in range(4)
                    ]
                    for kp in range(K_PAIRS):
                        for nn in range(4):
                            n = 4 * half + nn
                            nc.tensor.matmul(
                                psums[nn][:],
                                xe[:, 2 * kp : 2 * kp + 2, n * P : (n + 1) * P],
                                yt[:, 2 * kp : 2 * kp + 2, :],
                                start=(kp == 0),
                                stop=(kp == K_PAIRS - 1),
                                perf_mode=DR,
                            )
                        pending = _drain(pending, 2)
                    for nn in range(4):
                        n = 4 * half + nn
                        ot = out_pool.tile([P, M_CHUNK], BF16, name="ot")
                        # out = 0.875 - dot/2048  (exact before bf16 round)
                        if mc == M_CHUNKS - 1 and nn % 2 == 1:
                            # DVE is idle by the tail; splitting the final
                            # evictions drains the last blocks sooner.
                            nc.vector.tensor_scalar(
                                out=ot[:], in0=psums[nn][:],
                                scalar1=-1.0 / 2048.0, scalar2=0.875,
                                op0=ALU.mult, op1=ALU.add,
                            )
                        else:
                            nc.scalar.activation(
                                ot[:], psums[nn][:], ACTF.Copy,
                                bias=0.875, scale=-1.0 / 2048.0,
                            )
                        nc.sync.dma_start(out_d[n, mc], ot[:])
                _drain(pending, 4)
    nc.compile()
    return nc


_NC_CACHE = {}


def _get_nc():
    if "nc" not in _NC_CACHE:
        _NC_CACHE["nc"] = _build_bass()
    return _NC_CACHE["nc"]


def _pack_features(t: np.ndarray) -> np.ndarray:
    """(rows, 256) values -> fp8 [128, 2, rows]: out[p, h, i] = t[i, 128h+p]."""
    tt = np.ascontiguousarray(t.T)  # (256, rows)
    return np.ascontiguousarray(
        tt.reshape(D_HALVES, P, t.shape[0]).transpose(1, 0, 2)
    ).astype(ml_dtypes.float8_e4m3fn)


def _make_in_maps(x: np.ndarray, y: np.ndarray):
    yr = _pack_features(y)  # [128, 2, 8192] fp8
    y01 = np.ascontiguousarray(yr[:, :, : 2 * M_CHUNK])
    in_maps = []
    for i in range(N_CORES):
        xr = _pack_features(x[i * N_SH : (i + 1) * N_SH])  # [128, 2, 1024]
        in_maps.append({"xr": xr, "y01": y01, "yf": yr})
    return in_maps


def _deblock(blocked: np.ndarray) -> np.ndarray:
    # (N_TILES, M_CHUNKS, P, M_CHUNK) -> (N_SH, M)
    return np.ascontiguousarray(
        blocked.transpose(0, 2, 1, 3).reshape(N_SH, M)
    )


def kernel(x: np.ndarray, y: np.ndarray, _trace: bool = False):
    x = np.asarray(x, dtype=np.float32)
    y = np.asarray(y, dtype=np.float32)
    assert x.shape == (N, D) and y.shape == (M, D)

    nc = _get_nc()
    in_maps = _make_in_maps(x, y)
    res = run_bass_kernel_spmd(
        nc, in_maps, core_ids=list(range(N_CORES)), trace=_trace
    )
    out = np.concatenate(
        [_deblock(np.asarray(r["out"]).astype(np.float32)) for r in res.results],
        axis=0,
    )
    if _trace:
        return out, res
    return out
